# revision 7
# baseline (speedup 1.0000x reference)
"""DGCNN classifier forward pass on 8 Trainium2 NeuronCores.

Sharding: batch b = core//2, half h = core%2. Each core processes 2048 of the
4096 points of its batch element end-to-end; the full per-batch feature set
needed for KNN candidate search is reconstituted with pair-wise AllGathers.

EdgeConv trick: each EdgeConv here is a single 1x1 conv layer, so
  W @ [x_j - x_i; x_i] = Wa x_j + (Wb - Wa) x_i  (Wa = W[:, :C], Wb = W[:, C:])
and since BN is affine with gamma > 0 and LeakyReLU is monotone,
  max_k lrelu(g*(a_j + b_i) + beta) = lrelu(g*(max_k a_j + b_i) + beta).
So each EdgeConv = KNN top-32 + gather rows of a = X Wa^T + max-reduce.
"""
import sys
import os

sys.path.insert(0, '/opt/trn_rl_repo')

import numpy as np
import concourse.bass as bass
import concourse.mybir as mybir
from concourse import tile, library_config
from concourse.vector_clock import ScopedClock

dt = mybir.dt
AF = mybir.ActivationFunctionType
ALU = mybir.AluOpType

N = 4096          # points per cloud
NH = 2048         # points per core (half)
KNN = 32          # neighbors
NEG = -3.0e38

_patched = [False]
_split_ctr = [0]


def _patch_tile_drain():
    if _patched[0]:
        return
    _patched[0] = True

    def _patched_fn(self, tick_clock, wait_clock):
        nc = self.nc
        drain_inst = nc.sync.drain()
        wait_clock.add_sem_waits(
            drain_inst.ins, ScopedClock({None: tick_clock.global_clock}))
        si = drain_inst.ins.sync_info
        if si is not None and si.on_wait and len(si.on_wait) > 1:
            waits = list(si.on_wait)
            si.on_wait = waits[:1]
            for w in waits[1:]:
                extra = nc.sync.drain()
                esi = extra.ins.sync_info
                if esi is None:
                    extra.ins.sync_info = mybir.SyncInfo(on_wait=[w], on_update=[])
                else:
                    esi.on_wait = [w]
        nc.all_engine_barrier()
        assert self.sems is not None
        popped = nc._tile_sem_poison_stack.pop()
        assert popped is self._sem_poison
        nc.clear_and_free_semaphores(list(self.sems.allocated().values()))
        nc.all_engine_barrier()

    tile.TileContext._drain_and_barrier = _patched_fn


def _split_multi_waits(nc, limit=1):
    """This env's walrus rejects >1 sync-wait per instruction: hoist onto nops."""
    for fn in nc.m.functions:
        for bb in fn.blocks:
            new_insts = []
            for inst in bb.instructions:
                si = inst.sync_info
                if si is not None and si.on_wait and len(si.on_wait) > limit:
                    waits = list(si.on_wait)
                    for w in waits[:-limit]:
                        _split_ctr[0] += 1
                        nop = mybir.InstNoOp(
                            name=f"I-waitsplit-{_split_ctr[0]}", ins=[], outs=[])
                        nop.engine = inst.engine
                        nop.sync_info = mybir.SyncInfo(on_wait=[w], on_update=[])
                        new_insts.append(nop)
                    si.on_wait = waits[-limit:]
                new_insts.append(inst)
            bb.instructions[:] = new_insts


def _finalize(nc):
    from concourse.library_overlay import lower_extended_insts
    lower_extended_insts(nc)
    _split_multi_waits(nc)


# ---------------------------------------------------------------------------
# device program
# ---------------------------------------------------------------------------

def _build_kernel():
    _patch_tile_drain()
    nc = bass.Bass(target_bir_lowering=False, num_devices=8)

    f32 = dt.float32
    din = {}

    def inp(name, shape):
        din[name] = nc.dram_tensor(name, list(shape), f32, kind="ExternalInput")
        return din[name]

    xT_own = inp("xT_own", [3, NH])
    xT_full = inp("xT_full", [3, N])
    ident = inp("ident", [128, 128])
    eye9 = inp("eye9", [9, 1])
    # tnet convs
    tc_w = [inp(f"tc{i}_w", s) for i, s in enumerate([(3, 64), (64, 128), (128, 1024)], 1)]
    tc_g = [inp(f"tc{i}_g", (c, 1)) for i, c in enumerate([64, 128, 1024], 1)]
    tc_b = [inp(f"tc{i}_b", (c, 1)) for i, c in enumerate([64, 128, 1024], 1)]
    # tnet fcs (scale/bias pre-folded on host)
    tf_w = [inp(f"tf{i}_w", s) for i, s in enumerate([(1024, 512), (512, 256)], 1)]
    tf_sc = [inp(f"tf{i}_sc", (c, 1)) for i, c in enumerate([512, 256], 1)]
    tf_bi = [inp(f"tf{i}_bi", (c, 1)) for i, c in enumerate([512, 256], 1)]
    tout_w = inp("tout_w", [256, 9])
    tout_b = inp("tout_b", [9, 1])
    # edgeconvs
    EC_CIN = [3, 64, 64, 64]
    EC_COUT = [64, 64, 64, 128]
    eca_w = [inp(f"eca_w{i}", (EC_CIN[i], EC_COUT[i])) for i in range(4)]
    ecb_w = [inp(f"ecb_w{i}", (EC_CIN[i], EC_COUT[i])) for i in range(4)]
    ec_g = [inp(f"ec_g{i}", (EC_COUT[i], 1)) for i in range(4)]
    ec_b = [inp(f"ec_b{i}", (EC_COUT[i], 1)) for i in range(4)]
    # head
    HD = [(320, 1024), (1024, 512), (512, 128), (128, 40)]
    hd_w = [inp(f"hd_w{i}", s) for i, s in enumerate(HD)]
    hd_b = [inp(f"hd_b{i}", (s[1], 1)) for i, s in enumerate(HD)]

    out_logits = nc.dram_tensor("logits", [40, 1], f32, kind="ExternalOutput")
    out_inT = nc.dram_tensor("inT", [9, 1], f32, kind="ExternalOutput")

    with tile.TileContext(nc) as tc:
        with tc.tile_pool(name="persist", bufs=1) as pp, \
             tc.tile_pool(name="xaug", bufs=2) as pxo, \
             tc.tile_pool(name="xfull", bufs=2) as pxf, \
             tc.tile_pool(name="work", bufs=2) as wk, \
             tc.tile_pool(name="single", bufs=1) as sg, \
             tc.tile_pool(name="small", bufs=3) as sm, \
             tc.tile_pool(name="psum", bufs=6, space="PSUM") as psp, \
             tc.tile_pool(name="psum2", bufs=2, space="PSUM") as psp2, \
             tc.tile_pool(name="dram", bufs=2, space="DRAM") as dr:

            nc.gpsimd.load_library(library_config.mlp)
            nidx_reg = nc.gpsimd.to_reg(128 * KNN)

            t_ident = pp.tile([128, 128], f32)
            nc.sync.dma_start(t_ident[:], ident[:])

            def load_col(dram_t, rows):
                t = sm.tile([rows, 1], f32, tag="cols")
                nc.sync.dma_start(t[:], dram_t[0:rows, :])
                return t

            # ---------------- T-Net ----------------
            t_x_own = wk.tile([3, NH], f32, tag="bsb")
            t_x_full = wk.tile([3, N], f32, tag="gtx")
            nc.sync.dma_start(t_x_own[:], xT_own[:])
            nc.sync.dma_start(t_x_full[:], xT_full[:])

            w1 = sm.tile([3, 64], f32, tag="w1")
            nc.sync.dma_start(w1[:], tc_w[0][:])
            g1 = load_col(tc_g[0], 64)
            b1 = load_col(tc_b[0], 64)
            h1 = wk.tile([64, NH], f32, tag="asb")
            for j in range(NH // 512):
                p = psp.tile([128, 512], f32, tag="ps")
                nc.tensor.matmul(p[0:64, :], w1[:], t_x_own[:, j * 512:(j + 1) * 512],
                                 start=True, stop=True)
                nc.scalar.activation(h1[:, j * 512:(j + 1) * 512], p[0:64, :], AF.Relu,
                                     bias=b1[0:64, :], scale=g1[0:64, :])

            w2 = sm.tile([64, 128], f32, tag="w2")
            nc.sync.dma_start(w2[:], tc_w[1][:])
            g2 = load_col(tc_g[1], 128)
            b2 = load_col(tc_b[1], 128)
            h2 = wk.tile([128, NH], f32, tag="s")
            for j in range(NH // 512):
                p = psp.tile([128, 512], f32, tag="ps")
                nc.tensor.matmul(p[:], w2[:], h1[:, j * 512:(j + 1) * 512],
                                 start=True, stop=True)
                nc.scalar.activation(h2[:, j * 512:(j + 1) * 512], p[:], AF.Relu,
                                     bias=b2[:], scale=g2[:])

            gmax = pp.tile([128, 8], f32)
            for oc in range(8):
                w3 = sm.tile([128, 128], f32, tag="w3")
                nc.sync.dma_start(w3[:], tc_w[2][:, oc * 128:(oc + 1) * 128])
                g3 = sm.tile([128, 1], f32, tag="g3")
                b3 = sm.tile([128, 1], f32, tag="b3")
                nc.sync.dma_start(g3[:], tc_g[2][oc * 128:(oc + 1) * 128, :])
                nc.sync.dma_start(b3[:], tc_b[2][oc * 128:(oc + 1) * 128, :])
                rmax = sm.tile([128, 4], f32, tag="rmax")
                for j in range(NH // 512):
                    p = psp.tile([128, 512], f32, tag="ps")
                    nc.tensor.matmul(p[:], w3[:], h2[:, j * 512:(j + 1) * 512],
                                     start=True, stop=True)
                    nc.vector.tensor_reduce(out=rmax[:, j:j + 1], in_=p[:],
                                            op=ALU.max, axis=mybir.AxisListType.X)
                nc.vector.tensor_reduce(out=gmax[:, oc:oc + 1], in_=rmax[:],
                                        op=ALU.max, axis=mybir.AxisListType.X)
                # bn+relu after the max (monotone for g>0)
                nc.scalar.activation(gmax[:, oc:oc + 1], gmax[:, oc:oc + 1], AF.Relu,
                                     bias=b3[:], scale=g3[:])

            # pair-wise max of the global pool
            cc_in = dr.tile([128, 8], f32, tag="ccg")
            cc_out = dr.tile([128, 8], f32, tag="ccg2")
            nc.sync.dma_start(cc_in[:], gmax[:])
            nc.gpsimd.collective_compute(
                "AllReduce", ALU.max,
                replica_groups=[[0, 1], [2, 3], [4, 5], [6, 7]],
                ins=[cc_in.opt()], outs=[cc_out.opt()])
            nc.sync.dma_start(gmax[:], cc_out[:])

            # fcs
            def fc_chunked(x_cols, n_in, n_out, w_d, sc_d, bi_d, act):
                """x_cols: sbuf [128, n_in//128] (K-chunks in columns).
                returns sbuf [128, n_out//128]."""
                nk = n_in // 128
                nm = n_out // 128
                out = sm.tile([128, max(nm, 1)], f32, tag=f"fc{n_out}")
                for m in range(nm):
                    p = psp2.tile([128, 512], f32, tag="pfc")
                    for k in range(nk):
                        wkk = sm.tile([128, 128], f32, tag="fw")
                        nc.sync.dma_start(
                            wkk[:], w_d[k * 128:(k + 1) * 128, m * 128:(m + 1) * 128])
                        nc.tensor.matmul(p[:, 0:1], wkk[:], x_cols[:, k:k + 1],
                                         start=(k == 0), stop=(k == nk - 1))
                    sc = sm.tile([128, 1], f32, tag="fsc")
                    bi = sm.tile([128, 1], f32, tag="fbi")
                    nc.sync.dma_start(sc[:], sc_d[m * 128:(m + 1) * 128, :])
                    nc.sync.dma_start(bi[:], bi_d[m * 128:(m + 1) * 128, :])
                    nc.scalar.activation(out[:, m:m + 1], p[:, 0:1], act,
                                         bias=bi[:], scale=sc[:])
                return out

            f1 = fc_chunked(gmax, 1024, 512, tf_w[0], tf_sc[0], tf_bi[0], AF.Relu)
            f2 = fc_chunked(f1, 512, 256, tf_w[1], tf_sc[1], tf_bi[1], AF.Relu)

            # tout: [9,1] = tout_w.T @ f2(256) + tout_b + eye
            p9 = psp2.tile([128, 512], f32, tag="pfc")
            for k in range(2):
                wkk = sm.tile([128, 9], f32, tag="tw")
                nc.sync.dma_start(wkk[:], tout_w[k * 128:(k + 1) * 128, :])
                nc.tensor.matmul(p9[0:9, 0:1], wkk[:], f2[:, k:k + 1],
                                 start=(k == 0), stop=(k == 1))
            tb = sm.tile([9, 1], f32, tag="tb")
            te = sm.tile([9, 1], f32, tag="te")
            nc.sync.dma_start(tb[:], tout_b[:])
            nc.sync.dma_start(te[:], eye9[:])
            t9 = sm.tile([9, 1], f32, tag="t9")
            nc.scalar.activation(t9[:], p9[0:9, 0:1], AF.Identity, bias=tb[:])
            nc.vector.tensor_add(t9[:], t9[:], te[:])
            nc.sync.dma_start(out_inT[:], t9[:])

            # reshape t9 -> T [3,3] via dram roundtrip
            t9d = dr.tile([9, 1], f32, tag="t9d")
            nc.sync.dma_start(t9d[:], t9[:])
            tmat = sm.tile([3, 3], f32, tag="tmat")
            nc.sync.dma_start(tmat[:], t9d[:].rearrange("(a b) c -> a (b c)", a=3, b=3))

            # x0 = x @ T  (transposed: x0T = T^T @ xT = matmul(lhsT=T, rhs=xT))
            # engines can only address partition bases 0/32/64/96, so the
            # augmentation row sits at partition 32 for cin=3 (rows 3..31
            # zeroed) and at partition cin otherwise.
            def aug_row(cin):
                return 32 if cin == 3 else cin

            def make_aug(cin, ncols, tag):
                t = (pxo if ncols == NH else pxf).tile(
                    [aug_row(cin) + 1, ncols], f32, tag=tag)
                if cin == 3:
                    nc.vector.memset(t[:], 0.0)
                return t

            x0_own = make_aug(3, NH, "xo")
            for j in range(NH // 512):
                p = psp.tile([128, 512], f32, tag="ps")
                nc.tensor.matmul(p[0:3, :], tmat[:], t_x_own[:, j * 512:(j + 1) * 512],
                                 start=True, stop=True)
                nc.scalar.activation(x0_own[0:3, j * 512:(j + 1) * 512], p[0:3, :],
                                     AF.Identity)
            nc.vector.memset(x0_own[32:33, :], 1.0)

            ones_k = sm.tile([128, 1], f32, tag="ones")
            nc.vector.memset(ones_k[:], 1.0)

            def make_full_aug(src_cb, cin, tag):
                """src_cb(j) -> writes rows [0:cin] of column chunk j (512 wide)
                into a new [cin+1, N] tile; row cin = -0.5*sum(sq)."""
                xf = make_aug(cin, N, tag)
                ar = aug_row(cin)
                for j in range(N // 512):
                    src_cb(xf, j)
                for j in range(N // 512):
                    sq = wk.tile([128, 512], f32, tag="sq")
                    nc.scalar.activation(sq[0:cin, :], xf[0:cin, j * 512:(j + 1) * 512],
                                         AF.Square)
                    p1 = psp.tile([128, 512], f32, tag="ps")
                    nc.tensor.matmul(p1[0:1, :], ones_k[0:cin, :], sq[0:cin, :],
                                     start=True, stop=True)
                    nc.scalar.activation(xf[ar:ar + 1, j * 512:(j + 1) * 512],
                                         p1[0:1, :], AF.Identity, scale=-0.5)
                return xf

            def x0full_cb(xf, j):
                p = psp.tile([128, 512], f32, tag="ps")
                nc.tensor.matmul(p[0:3, :], tmat[:], t_x_full[:, j * 512:(j + 1) * 512],
                                 start=True, stop=True)
                nc.scalar.activation(xf[0:3, j * 512:(j + 1) * 512], p[0:3, :],
                                     AF.Identity)

            x_own = x0_own
            x_full = make_full_aug(x0full_cb, 3, "xf")

            pm_tiles = []  # per-EC partial max [cout, 1]

            for ec in range(4):
                cin, cout = EC_CIN[ec], EC_COUT[ec]
                wa = sm.tile([cin, cout], f32, tag="wa")
                wb = sm.tile([cin, cout], f32, tag="wb")
                nc.sync.dma_start(wa[:], eca_w[ec][:])
                nc.sync.dma_start(wb[:], ecb_w[ec][:])
                gg = load_col(ec_g[ec], cout)
                bb = load_col(ec_b[ec], cout)

                # a-table -> DRAM  [N, cout] (two halves)
                a_dram = dr.tile([N, cout], f32, tag="a_dram")
                for hh in range(2):
                    a_sb = wk.tile([128, 16 * cout], f32, tag="asb")
                    for rc in range(16):
                        rcg = hh * 16 + rc
                        pA = psp.tile([128, 512], f32, tag="ps")
                        nc.tensor.matmul(pA[:, 0:cout],
                                         x_full[0:cin, rcg * 128:(rcg + 1) * 128],
                                         wa[:], start=True, stop=True)
                        nc.scalar.activation(a_sb[:, rc * cout:(rc + 1) * cout],
                                             pA[:, 0:cout], AF.Identity)
                    nc.sync.dma_start(
                        a_dram[hh * 2048:(hh + 1) * 2048, :].rearrange(
                            "(rc p) c -> p rc c", rc=16, p=128),
                        a_sb[:].rearrange("p (rc c) -> p rc c", rc=16, c=cout))

                # b-table (own rows) in SBUF
                b_sb = wk.tile([128, 16 * cout], f32, tag="bsb")
                for tch in range(16):
                    pB = psp.tile([128, 512], f32, tag="ps")
                    nc.tensor.matmul(pB[:, 0:cout],
                                     x_own[0:cin, tch * 128:(tch + 1) * 128],
                                     wb[:], start=True, stop=True)
                    nc.scalar.activation(b_sb[:, tch * cout:(tch + 1) * cout],
                                         pB[:, 0:cout], AF.Identity)

                if ec < 3:
                    nxt_own = make_aug(cout, NH, "xo")
                    nc.vector.memset(nxt_own[aug_row(cout):aug_row(cout) + 1, :], 1.0)
                else:
                    nxt_own = sg.tile([cout, NH], f32, tag="xo4")

                for t in range(16):
                    s = wk.tile([128, N], f32, tag="s")
                    for j in range(N // 512):
                        p = psp.tile([128, 512], f32, tag="ps")
                        nc.tensor.matmul(p[:], x_own[:, t * 128:(t + 1) * 128],
                                         x_full[:, j * 512:(j + 1) * 512],
                                         start=True, stop=True)
                        nc.scalar.activation(s[:, j * 512:(j + 1) * 512], p[:],
                                             AF.Identity, scale=2.0)
                    idx = sm.tile([128, KNN], dt.uint16, tag="idx")
                    for r in range(4):
                        v8 = sm.tile([128, 8], f32, tag="v8")
                        nc.vector.max(v8[:], s[:])
                        nc.vector.max_index(idx[:, r * 8:(r + 1) * 8], v8[:], s[:])
                        if r < 3:
                            nc.vector.match_replace(s[:], v8[:], s[:], NEG)

                    # wrapped idx layout for dma_gather via dram roundtrip
                    idxd = dr.tile([128 * KNN], dt.uint16, tag="idxd")
                    nc.sync.dma_start(
                        idxd[:].rearrange("(r k) -> r k", r=128, k=KNN), idx[:])
                    tw = sm.tile([128, 256], dt.int16, tag="tw_idx")
                    src_view = idxd[:].bitcast(dt.int16).rearrange(
                        "(j p k) -> p k j", j=8, p=16, k=KNN)
                    for q in range(8):
                        nc.sync.dma_start(
                            tw[16 * q:16 * (q + 1), :].rearrange(
                                "p (k j) -> p k j", k=KNN, j=8),
                            src_view)

                    gt = wk.tile([128, KNN, cout], f32, tag="gtx")
                    nc.gpsimd.dma_gather(
                        out_ap=gt[:], in_ap=a_dram[:], idxs_ap=tw[:],
                        num_idxs=128 * KNN, num_idxs_reg=nidx_reg,
                        elem_size=cout, single_packet=False)

                    m = sm.tile([128, cout], f32, tag="m")
                    nc.vector.tensor_reduce(
                        out=m[:], in_=gt[:].rearrange("p k c -> p c k"),
                        op=ALU.max, axis=mybir.AxisListType.X)
                    nc.vector.tensor_add(m[:], m[:],
                                         b_sb[:, t * cout:(t + 1) * cout])
                    pT = psp.tile([128, 512], f32, tag="ps")
                    nc.tensor.transpose(pT[0:cout, 0:128], m[:, 0:cout], t_ident[:])
                    mt = sm.tile([cout, 128], f32, tag="mt")
                    nc.scalar.activation(mt[:], pT[0:cout, 0:128], AF.Identity,
                                         bias=bb[0:cout, :], scale=gg[0:cout, :])
                    nc.vector.scalar_tensor_tensor(
                        nxt_own[0:cout, t * 128:(t + 1) * 128], mt[:], 0.01, mt[:],
                        op0=ALU.mult, op1=ALU.max)

                # partial max over own cols for the head
                pm = pp.tile([cout, 1], f32, tag=f"pm{ec}")
                nc.vector.tensor_reduce(out=pm[:], in_=nxt_own[0:cout, :],
                                        op=ALU.max, axis=mybir.AxisListType.X)
                pm_tiles.append(pm)

                x_own = nxt_own
                if ec < 3:
                    # exchange halves -> full [cout, N], then augment with -sq/2
                    cc1 = dr.tile([cout, NH], f32, tag="cc1")
                    cc2 = dr.tile([2, cout, NH], f32, tag="cc2")
                    nc.sync.dma_start(cc1[:], nxt_own[0:cout, :])
                    nc.gpsimd.collective_compute(
                        "AllGather", ALU.bypass,
                        replica_groups=[[0, 1], [2, 3], [4, 5], [6, 7]],
                        ins=[cc1.opt()], outs=[cc2.opt()])

                    def nxt_cb(xf, j, cc2=cc2, cout=cout):
                        h = (j * 512) // NH
                        o = (j * 512) % NH
                        nc.sync.dma_start(
                            xf[0:cout, j * 512:(j + 1) * 512],
                            cc2[h, :, o:o + 512])

                    x_full = make_full_aug(nxt_cb, cout, "xf")

            # ---------------- head ----------------
            # g320 chunks: [x1(64)|x2(64)], [x3(64)|x4(0:64)], [x4(64:128)]
            gch = sm.tile([128, 3], f32, tag="g320")
            nc.sync.dma_start(gch[0:64, 0:1], pm_tiles[0][:])
            nc.sync.dma_start(gch[64:128, 0:1], pm_tiles[1][:])
            nc.sync.dma_start(gch[0:64, 1:2], pm_tiles[2][:])
            nc.sync.dma_start(gch[64:128, 1:2], pm_tiles[3][0:64, :])
            nc.sync.dma_start(gch[0:64, 2:3], pm_tiles[3][64:128, :])
            nc.vector.memset(gch[64:128, 2:3], NEG)

            ccg1 = dr.tile([128, 3], f32, tag="cch")
            ccg2 = dr.tile([128, 3], f32, tag="cch2")
            nc.sync.dma_start(ccg1[:], gch[:])
            nc.gpsimd.collective_compute(
                "AllReduce", ALU.max,
                replica_groups=[[0, 1], [2, 3], [4, 5], [6, 7]],
                ins=[ccg1.opt()], outs=[ccg2.opt()])
            nc.sync.dma_start(gch[:], ccg2[:])

            def head_fc(x_cols, kchunks, n_out, w_d, b_d, last):
                nm = (n_out + 127) // 128
                out = sm.tile([128, max(nm, 1)], f32, tag=f"hd{n_out}")
                for m in range(nm):
                    mw = min(128, n_out - m * 128)
                    p = psp2.tile([128, 512], f32, tag="pfc")
                    for ki, (kb, ksz, col) in enumerate(kchunks):
                        wkk = sm.tile([128, 128], f32, tag="hw")
                        nc.sync.dma_start(
                            wkk[0:ksz, 0:mw],
                            w_d[kb:kb + ksz, m * 128:m * 128 + mw])
                        nc.tensor.matmul(p[0:mw, 0:1], wkk[0:ksz, 0:mw],
                                         x_cols[0:ksz, col:col + 1],
                                         start=(ki == 0),
                                         stop=(ki == len(kchunks) - 1))
                    bi = sm.tile([128, 1], f32, tag="hbi")
                    nc.sync.dma_start(bi[0:mw, :], b_d[m * 128:m * 128 + mw, :])
                    nc.scalar.activation(out[0:mw, m:m + 1], p[0:mw, 0:1],
                                         AF.Identity, bias=bi[0:mw, :])
                    if not last:
                        nc.vector.scalar_tensor_tensor(
                            out[0:mw, m:m + 1], out[0:mw, m:m + 1], 0.01,
                            out[0:mw, m:m + 1], op0=ALU.mult, op1=ALU.max)
                return out

            hh1 = head_fc(gch, [(0, 128, 0), (128, 128, 1), (256, 64, 2)],
                          1024, hd_w[0], hd_b[0], False)
            hh2 = head_fc(hh1, [(k * 128, 128, k) for k in range(8)],
                          512, hd_w[1], hd_b[1], False)
            hh3 = head_fc(hh2, [(k * 128, 128, k) for k in range(4)],
                          128, hd_w[2], hd_b[2], False)
            p40 = psp2.tile([128, 512], f32, tag="pfc")
            w40 = sm.tile([128, 40], f32, tag="w40")
            nc.sync.dma_start(w40[:], hd_w[3][:])
            nc.tensor.matmul(p40[0:40, 0:1], w40[:], hh3[:, 0:1],
                             start=True, stop=True)
            b40 = sm.tile([40, 1], f32, tag="b40")
            nc.sync.dma_start(b40[:], hd_b[3][:])
            lg = sm.tile([40, 1], f32, tag="lg")
            nc.scalar.activation(lg[:], p40[0:40, 0:1], AF.Identity, bias=b40[:])
            nc.sync.dma_start(out_logits[:], lg[:])

    _finalize(nc)
    return nc


_NC_CACHE = [None]


def _get_nc():
    if _NC_CACHE[0] is None:
        _NC_CACHE[0] = _build_kernel()
    return _NC_CACHE[0]


# ---------------------------------------------------------------------------
# host side
# ---------------------------------------------------------------------------

def _np(a):
    return np.asarray(a, dtype=np.float32)


def _prep_inputs(x, params):
    """Build the 8 per-core input maps."""
    x = _np(x)                      # [4, 4096, 3]
    p = params

    com = {}
    com["ident"] = np.eye(128, dtype=np.float32)
    com["eye9"] = np.eye(3, dtype=np.float32).reshape(9, 1)
    for i, (W, g, b) in enumerate(p["tnet"]["convs"], 1):
        com[f"tc{i}_w"] = _np(W).T.copy()
        com[f"tc{i}_g"] = _np(g).reshape(-1, 1)
        com[f"tc{i}_b"] = _np(b).reshape(-1, 1)
    for i, (W, bb, g, be) in enumerate(p["tnet"]["fcs"], 1):
        com[f"tf{i}_w"] = _np(W).T.copy()
        com[f"tf{i}_sc"] = _np(g).reshape(-1, 1)
        com[f"tf{i}_bi"] = (_np(g) * _np(bb) + _np(be)).reshape(-1, 1)
    W3, b3 = p["tnet"]["out"]
    com["tout_w"] = _np(W3).T.copy()
    com["tout_b"] = _np(b3).reshape(-1, 1)
    for i, key in enumerate(["ec1", "ec2", "ec3", "ec4"]):
        (W, g, b) = p[key][0]
        W = _np(W)
        cin = W.shape[1] // 2
        Wa = W[:, :cin]
        Wb = W[:, cin:]
        com[f"eca_w{i}"] = Wa.T.copy()
        com[f"ecb_w{i}"] = (Wb - Wa).T.copy()
        com[f"ec_g{i}"] = _np(g).reshape(-1, 1)
        com[f"ec_b{i}"] = _np(b).reshape(-1, 1)
    for i, (W, bb) in enumerate(p["head"]):
        com[f"hd_w{i}"] = _np(W).T.copy()
        com[f"hd_b{i}"] = _np(bb).reshape(-1, 1)

    in_maps = []
    for core in range(8):
        b = core // 2
        h = core % 2
        m = dict(com)
        xb = x[b].T.copy()          # [3, 4096]
        m["xT_full"] = np.ascontiguousarray(xb)
        m["xT_own"] = np.ascontiguousarray(xb[:, h * NH:(h + 1) * NH])
        in_maps.append(m)
    return in_maps


_LAST_RESULTS = [None]


def kernel(x, params):
    from concourse.bass_utils import run_bass_kernel_spmd
    nc = _get_nc()
    in_maps = _prep_inputs(x, params)
    trace = bool(int(os.environ.get("DGCNN_TRACE", "0")))
    res = run_bass_kernel_spmd(nc, in_maps, core_ids=list(range(8)), trace=trace)
    _LAST_RESULTS[0] = res
    logits = np.stack([res.results[2 * b]["logits"][:, 0] for b in range(4)])
    inT = np.stack([res.results[2 * b]["inT"][:, 0].reshape(3, 3) for b in range(4)])
    return logits.astype(np.float32), inT.astype(np.float32)
